# revision 30
# baseline (speedup 1.0000x reference)
"""DeformGNN Trainium2 kernel.

Data-parallel over batch: 16 batches -> 8 cores x 2 batches each.
Per batch on device:
  - bilinear feature sampling via dma_gather from host-transposed
    features [HW, C] (indices/weights precomputed on host from base_point)
  - 14 graph convolutions h = relu(x @ Wa.T + adj @ (x @ Wb.T) + b)
    with the activation kept in transposed layout [S, N] so biases are
    per-partition, using float32r matmuls (full PE rate at N>=256)
  - fc head, mask multiply, laplacian energy
adj is pre-transposed on host (adjT[j, i] = adj[i, j]) so the PE
contraction (over j) runs on the partition axis with no on-device
transposes of the 4MB matrix.
"""
import numpy as np
import concourse.bass as bass
import concourse.tile as tile
from concourse import bacc, mybir, masks
from concourse.bass_utils import run_bass_kernel_spmd

F32 = mybir.dt.float32
F32R = mybir.dt.float32r
I16 = mybir.dt.int16

B, C, H, W, N = 16, 128, 128, 128, 1024
HW = H * W
S = 256
NMID = 6
NG = 2 + 2 * NMID  # 14 gconvs
NCORES = 8
PER = B // NCORES  # batches per core
NJ = N // 128      # 8 n-chunks
EPS2 = 2e-10       # (dx^2+eps)+(dy^2+eps) = dx^2+dy^2+2eps

ACT_F = mybir.ActivationFunctionType


def build(reps: int = 0, n_swdge: int = 4, do_gather: bool = True,
          n_blocks: int = NMID, do_tail: bool = True, timing: bool = False,
          out_eng: str = "sync", w_eng: str = "scalar",
          stream_w: bool = True):
    """Build the per-core Bass program. reps>0 wraps the batch loop in a
    hardware For_i repeat loop (for differential timing only). The other
    kwargs disable phases for profiling bisection. timing=True turns the
    bulky inputs (features/adj/weights) into uninitialized Internal DRAM so
    timing runs ship almost no data over the tunnel."""
    nc = bacc.Bacc("TRN2", num_swdge_queues=n_swdge)
    oeng = getattr(nc, out_eng)
    weng_name = w_eng
    bulk = "Internal" if timing else "ExternalInput"

    # ---- DRAM I/O (per core) ----
    featT = nc.dram_tensor("featT", [PER, HW, C], F32, kind=bulk)
    adjT_d = nc.dram_tensor("adjT", [PER, N, N], F32R, kind=bulk)
    idx_d = nc.dram_tensor("idx", [PER, 4, 128, N // 16], I16, kind="ExternalInput")
    bw_d = nc.dram_tensor("bw", [PER, 4, 128, NJ], F32, kind="ExternalInput")
    xyT_d = nc.dram_tensor("xyT", [PER, 2, N], F32R, kind="ExternalInput")
    mask2_d = nc.dram_tensor("mask2", [PER, 2, N], F32, kind="ExternalInput")
    wfirst_d = nc.dram_tensor("wfirst", [2, 130, S], F32R, kind=bulk)
    wmid_d = nc.dram_tensor("wmid", [NMID, 4, S, S], F32R, kind=bulk)
    wlast_d = nc.dram_tensor("wlast", [2, S, S], F32R, kind=bulk)
    wfc_d = nc.dram_tensor("wfc", [S, 2], F32R, kind="ExternalInput")
    bias_d = nc.dram_tensor("biases", [NG, 128, 2], F32, kind="ExternalInput")
    fcb_d = nc.dram_tensor("fcb", [2, 1], F32, kind="ExternalInput")
    ones2_d = nc.dram_tensor("ones2", [2, 1], F32R, kind="ExternalInput")

    predT_o = nc.dram_tensor("predT", [PER, 2, N], F32, kind="ExternalOutput")
    polyT_o = nc.dram_tensor("polyT", [PER, 2, N], F32, kind="ExternalOutput")
    energy_o = nc.dram_tensor("energy", [PER, 1], F32, kind="ExternalOutput")

    with tile.TileContext(nc) as tc:
        with (
            tc.tile_pool(name="const", bufs=1) as cpool,
            tc.tile_pool(name="act", bufs=3) as apool,
            tc.tile_pool(name="z1p", bufs=1) as zpool,
            tc.tile_pool(name="gat", bufs=1) as gpool,
            tc.tile_pool(name="cnT", bufs=2) as cnpool,
            tc.tile_pool(name="adj", bufs=(2 if stream_w else 1)) as adjpool,
            tc.tile_pool(name="wst", bufs=2) as wpool,
            tc.tile_pool(name="sml", bufs=1) as spool,
            tc.tile_pool(name="hps", bufs=1, space="PSUM") as hpool,
            tc.tile_pool(name="zps", bufs=2, space="PSUM") as zppool,
            tc.tile_pool(name="tps", bufs=2, space="PSUM") as tppool,
        ):
            # ---- constants & weights (once) ----
            ident = cpool.tile([128, 128], F32)
            masks.make_identity(nc, ident[:])
            eps_t = cpool.tile([1, 1], F32)
            nc.gpsimd.memset(eps_t[:], EPS2)
            ones2 = cpool.tile([2, 1], F32R)
            nc.sync.dma_start(ones2[:], ones2_d[:])
            fcb_t = cpool.tile([2, 1], F32)
            nc.sync.dma_start(fcb_t[:], fcb_d[:])
            bias_t = cpool.tile([128, NG, 2], F32)
            nc.sync.dma_start(bias_t[:], bias_d[:].rearrange("g p c -> p g c"))

            # weights ride their own HWDGE FIFO so they never block the
            # small per-batch control DMAs
            weng = getattr(nc, weng_name)
            wfa = cpool.tile([128, 2, S], F32R)  # first-layer d-rows 0:128
            weng.dma_start(wfa[:], wfirst_d[:, 0:128, :].rearrange("k p s -> p k s"))
            wfb = cpool.tile([2, 2, S], F32R)    # first-layer d-rows 128:130 (xy)
            weng.dma_start(wfb[:], wfirst_d[:, 128:130, :].rearrange("k p s -> p k s"))
            if not stream_w:
                wmid = cpool.tile([128, NMID, 4, 2, S], F32R)
                for i in range(NMID):
                    weng.dma_start(
                        wmid[:, i], wmid_d[i].rearrange("k (d p) s -> p k d s", p=128))
            wlast = cpool.tile([128, 2, 2, S], F32R)
            weng.dma_start(wlast[:], wlast_d[:].rearrange("k (d p) s -> p k d s", p=128))
            wfc = cpool.tile([128, 2, 2], F32R)
            weng.dma_start(wfc[:], wfc_d[:].rearrange("(d p) c -> p d c", p=128))

            def gconv(g, xchunks, wa_chunks, wb_chunks, outT, prevT):
                """h = x @ Wa.T + adj @ (x @ Wb.T) (+bias, +prevT residual, relu)
                xchunks: list of (AP [dp, N]) activation d-chunks (f32r)
                wa/wb_chunks: list of AP [dp, S] (f32r), same chunking
                outT: [128, 2*N] flat f32r target ([s-chunk sigma at sigma*N], [i])
                prevT: same layout or None -> relu(h+bias) ; else relu(h+bias+prevT)
                """
                nd = len(xchunks)
                # z1 = x @ Wb.T  (natural [n, s]), into SBUF f32r
                z1 = zpool.tile([128, NJ * S], F32R, tag="z1")
                for jp in range(NJ // 2):
                    zps = zppool.tile([128, 512], F32, tag="zp")
                    for jj in range(2):
                        j = 2 * jp + jj
                        for d in range(nd):
                            nc.tensor.matmul(
                                zps[:, jj * 256:(jj + 1) * 256],
                                xchunks[d][:, j * 128:(j + 1) * 128],
                                wb_chunks[d],
                                start=(d == 0), stop=(d == nd - 1))
                    eng = nc.vector if jp % 2 == 0 else nc.scalar
                    if eng is nc.vector:
                        nc.vector.tensor_copy(z1[:, jp * 512:(jp + 1) * 512], zps[:])
                    else:
                        nc.scalar.activation(z1[:, jp * 512:(jp + 1) * 512], zps[:],
                                             ACT_F.Copy)
                # hT[sigma, i] accumulation: z0T then agg
                hps = [hpool.tile([128, 512], F32, tag=f"h{si}", name=f"h{si}")
                       for si in range(4)]
                for si in range(2):
                    for hh in range(2):
                        for d in range(nd):
                            nc.tensor.matmul(
                                hps[si * 2 + hh][:],
                                wa_chunks[d][:, si * 128:(si + 1) * 128],
                                xchunks[d][:, hh * 512:(hh + 1) * 512],
                                start=(d == 0), stop=False)
                for j in range(NJ):
                    # si-outer keeps the two MMs sharing a stationary adjacent
                    for si in range(2):
                        for hh in range(2):
                            nc.tensor.matmul(
                                hps[si * 2 + hh][:],
                                z1[:, j * 256 + si * 128: j * 256 + (si + 1) * 128],
                                adjT_sb[:, j, hh * 512:(hh + 1) * 512],
                                start=False, stop=(j == NJ - 1))
                # activation: outT = relu(h + bias (+prev))
                for si in range(2):
                    for hh in range(2):
                        dst = outT[:, si * N + hh * 512: si * N + (hh + 1) * 512]
                        if prevT is None:
                            nc.scalar.activation(dst, hps[si * 2 + hh][:], ACT_F.Relu,
                                                 bias=bias_t[:, g, si:si + 1])
                        else:
                            nc.vector.scalar_tensor_tensor(
                                dst, hps[si * 2 + hh][:], bias_t[:, g, si:si + 1],
                                prevT[:, si * N + hh * 512: si * N + (hh + 1) * 512],
                                op0=mybir.AluOpType.add, op1=mybir.AluOpType.add)
                            nc.scalar.activation(dst, dst, ACT_F.Relu)

            def batch_body(b):
                nonlocal adjT_sb
                # small control DMAs first on the sync FIFO, then the 4MB adjT
                idx_t = spool.tile([128, 4, N // 16], I16, tag="idx", bufs=2)
                nc.gpsimd.dma_start(idx_t[:], idx_d[b].rearrange("k p c -> p k c"))
                bw_t = spool.tile([128, 4, NJ], F32, tag="bw", bufs=2)
                nc.sync.dma_start(bw_t[:], bw_d[b].rearrange("k p c -> p k c"))
                xyT = spool.tile([2, N], F32R, tag="xyT", bufs=2)
                nc.sync.dma_start(xyT[:], xyT_d[b])
                mask2 = spool.tile([2, N], F32, tag="mask2", bufs=2)
                nc.sync.dma_start(mask2[:], mask2_d[b])
                # adjacency (transposed on host): [p(j), jchunk, i]
                adjT_sb = adjpool.tile([128, NJ, N], F32R, tag="adjT")
                nc.sync.dma_start(
                    adjT_sb[:], adjT_d[b].rearrange("(j p) i -> p j i", p=128))

                cnnT = cnpool.tile([128, N], F32R, tag="cnnT")
                if do_gather:
                    g_t = [gpool.tile([128, NJ, C], F32, tag=f"g{k}", name=f"g{k}")
                           for k in range(4)]
                    for k in range(4):
                        nc.gpsimd.dma_gather(g_t[k][:], featT[b], idx_t[:, k, :],
                                             N, N, C, queue_num=k % n_swdge)
                    # cnn[:, c, :] = sum_k bw[k][c] * g_k[:, c, :] (ping-pong A/cnn)
                    tmpA = gpool.tile([128, NJ, C], F32, tag="tmpA")
                    cnn = gpool.tile([128, NJ, C], F32, tag="cnn")
                    for cch in range(NJ):
                        nc.vector.tensor_scalar_mul(
                            tmpA[:, cch, :], g_t[0][:, cch, :], bw_t[:, 0, cch:cch + 1])
                        nc.vector.scalar_tensor_tensor(
                            cnn[:, cch, :], g_t[1][:, cch, :], bw_t[:, 1, cch:cch + 1],
                            tmpA[:, cch, :], op0=mybir.AluOpType.mult,
                            op1=mybir.AluOpType.add)
                        nc.vector.scalar_tensor_tensor(
                            tmpA[:, cch, :], g_t[2][:, cch, :], bw_t[:, 2, cch:cch + 1],
                            cnn[:, cch, :], op0=mybir.AluOpType.mult,
                            op1=mybir.AluOpType.add)
                        nc.vector.scalar_tensor_tensor(
                            cnn[:, cch, :], g_t[3][:, cch, :], bw_t[:, 3, cch:cch + 1],
                            tmpA[:, cch, :], op0=mybir.AluOpType.mult,
                            op1=mybir.AluOpType.add)
                    # transpose cnn -> cnnT [c, n] f32r
                    for half in range(2):
                        tp = tppool.tile([128, 512], F32, tag="tp")
                        for q in range(4):
                            cch = half * 4 + q
                            nc.tensor.transpose(tp[:, q * 128:(q + 1) * 128],
                                                cnn[:, cch, :], ident[:])
                        nc.scalar.activation(cnnT[:, half * 512:(half + 1) * 512],
                                             tp[:], ACT_F.Copy)
                else:
                    # profiling stand-in: any f32r-produced data
                    nc.vector.tensor_copy(cnnT[:], adjT_sb[:, 0, :])

                # ---- gconv stack (transposed activations) ----
                xA = apool.tile([128, 2 * N], F32R, tag="act")
                gconv(0, [cnnT[:], xyT[:]],
                      [wfa[:, 0], wfb[:, 0]], [wfa[:, 1], wfb[:, 1]], xA, None)
                prev = xA
                for i in range(n_blocks):
                    if stream_w:
                        wblk = wpool.tile([128, 4, 2, S], F32R, tag="wblk")
                        weng.dma_start(
                            wblk[:], wmid_d[i].rearrange("k (d p) s -> p k d s", p=128))
                    else:
                        wblk = wmid[:, i]
                    o1 = apool.tile([128, 2 * N], F32R, tag="act")
                    gconv(1 + 2 * i, [prev[:, 0:N], prev[:, N:2 * N]],
                          [wblk[:, 0, 0], wblk[:, 0, 1]],
                          [wblk[:, 1, 0], wblk[:, 1, 1]], o1, None)
                    o2 = apool.tile([128, 2 * N], F32R, tag="act")
                    gconv(2 + 2 * i, [o1[:, 0:N], o1[:, N:2 * N]],
                          [wblk[:, 2, 0], wblk[:, 2, 1]],
                          [wblk[:, 3, 0], wblk[:, 3, 1]], o2, prev)
                    prev = o2
                out = apool.tile([128, 2 * N], F32R, tag="act")
                gconv(NG - 1, [prev[:, 0:N], prev[:, N:2 * N]],
                      [wlast[:, 0, 0], wlast[:, 0, 1]],
                      [wlast[:, 1, 0], wlast[:, 1, 1]], out, None)

                if not do_tail:
                    # profiling builds: just flush something observable
                    dummy = spool.tile([2, N], F32, tag="predT", name="dummy")
                    nc.vector.tensor_copy(dummy[:], out[0:2, 0:N].bitcast(F32))
                    oeng.dma_start(predT_o[b], dummy[:])
                    return
                # ---- fc head: predT[2, N] = (out.T @ fcW.T).T + fcb ----
                predT = spool.tile([2, N], F32, tag="predT")
                for hh in range(2):
                    fps = tppool.tile([128, 512], F32, tag="tp")
                    for d in range(2):
                        nc.tensor.matmul(
                            fps[0:2, :], wfc[:, d, :],
                            out[:, d * N + hh * 512: d * N + (hh + 1) * 512],
                            start=(d == 0), stop=(d == 1))
                    nc.vector.tensor_scalar_add(
                        predT[:, hh * 512:(hh + 1) * 512], fps[0:2, :], fcb_t[:])
                oeng.dma_start(predT_o[b], predT[:])

                # delta = pred * mask ; poly = base_point + delta
                deltaT = spool.tile([2, N], F32, tag="deltaT")
                nc.vector.tensor_mul(deltaT[:], predT[:], mask2[:])
                polyT = spool.tile([2, N], F32, tag="polyT")
                nc.vector.tensor_add(polyT[:], deltaT[:], xyT[:].bitcast(F32))
                oeng.dma_start(polyT_o[b], polyT[:])

                # laplacian energy: e = mean_n sqrt(|d - adj@d|^2 + 2eps), d = delta
                tpd = tppool.tile([128, 512], F32, tag="tp")
                for cch in range(NJ):
                    nc.tensor.transpose(tpd[:, cch * 2:(cch + 1) * 2],
                                        deltaT[:, cch * 128:(cch + 1) * 128],
                                        ident[0:2, 0:2])
                dnat = spool.tile([128, NJ * 2], F32R, tag="dnat")
                nc.vector.tensor_copy(dnat[:], tpd[:, 0:NJ * 2])
                diffT = spool.tile([2, N], F32, tag="diffT")
                for hh in range(2):
                    aps = tppool.tile([128, 512], F32, tag="tp")
                    for j in range(NJ):
                        nc.tensor.matmul(
                            aps[0:2, :], dnat[:, j * 2:(j + 1) * 2],
                            adjT_sb[:, j, hh * 512:(hh + 1) * 512],
                            start=(j == 0), stop=(j == NJ - 1))
                    nc.vector.scalar_tensor_tensor(
                        diffT[:, hh * 512:(hh + 1) * 512], aps[0:2, :], -1.0,
                        deltaT[:, hh * 512:(hh + 1) * 512],
                        op0=mybir.AluOpType.mult, op1=mybir.AluOpType.add)
                sq = spool.tile([2, N], F32R, tag="sq")
                nc.vector.tensor_mul(sq[:], diffT[:], diffT[:])
                sqrtv = spool.tile([1, N], F32, tag="sqrtv")
                for hh in range(2):
                    sps = tppool.tile([128, 512], F32, tag="tp")
                    nc.tensor.matmul(sps[0:1, :], ones2[:],
                                     sq[:, hh * 512:(hh + 1) * 512])
                    nc.scalar.activation(sqrtv[:, hh * 512:(hh + 1) * 512],
                                         sps[0:1, :], ACT_F.Sqrt, bias=eps_t[:])
                esum = spool.tile([1, 1], F32, tag="esum")
                nc.vector.reduce_sum(esum[:], sqrtv[:], axis=mybir.AxisListType.X)
                en = spool.tile([1, 1], F32, tag="en")
                nc.scalar.mul(en[:], esum[:], 1.0 / N)
                oeng.dma_start(energy_o[b:b + 1, :], en[:])

            adjT_sb = None
            if reps:
                with tc.For_i(0, reps, 1):
                    for b in range(PER):
                        batch_body(b)
            else:
                for b in range(PER):
                    batch_body(b)

    nc.compile()
    return nc


# ---------------- host side ----------------

def prep_host(inputs):
    """Shard + lay out inputs for the 8 cores. Returns list of in_maps."""
    features = np.ascontiguousarray(np.asarray(inputs["features"], np.float32))
    bp = np.asarray(inputs["base_point"], np.float32)
    adj = np.asarray(inputs["base_normalized_point_adjacent"], np.float32)
    mask = np.asarray(inputs["base_point_mask"], np.float32)
    first_W = np.asarray(inputs["first_W"], np.float32)
    first_b = np.asarray(inputs["first_b"], np.float32)
    mid_W = np.asarray(inputs["mid_W"], np.float32)
    mid_b = np.asarray(inputs["mid_b"], np.float32)
    last_W = np.asarray(inputs["last_W"], np.float32)
    last_b = np.asarray(inputs["last_b"], np.float32)
    fc_W = np.asarray(inputs["fc_W"], np.float32)
    fc_b = np.asarray(inputs["fc_b"], np.float32)

    featT = np.ascontiguousarray(features.reshape(B, C, HW).transpose(0, 2, 1))
    adjT = np.ascontiguousarray(adj.transpose(0, 2, 1))

    # bilinear corners (must mirror reference._interpolated_sum exactly, f32)
    Xs = bp[:, :, 0] * np.float32(W)
    Ys = bp[:, :, 1] * np.float32(H)
    X0 = np.floor(Xs); X1 = X0 + np.float32(1)
    Y0 = np.floor(Ys); Y1 = Y0 + np.float32(1)
    w00 = (X1 - Xs) * (Y1 - Ys)
    w01 = (X1 - Xs) * (Ys - Y0)
    w10 = (Xs - X0) * (Y1 - Ys)
    w11 = (Xs - X0) * (Ys - Y0)
    X0c = np.clip(X0, 0, W - 1).astype(np.int32)
    X1c = np.clip(X1, 0, W - 1).astype(np.int32)
    Y0c = np.clip(Y0, 0, H - 1).astype(np.int32)
    Y1c = np.clip(Y1, 0, H - 1).astype(np.int32)
    idx4 = np.stack([X0c + Y0c * W, X0c + Y1c * W,
                     X1c + Y0c * W, X1c + Y1c * W], axis=1).astype(np.int16)  # [B,4,N]
    w4 = np.stack([w00, w01, w10, w11], axis=1).astype(np.float32)            # [B,4,N]
    # wrap layouts
    idx_wrap = np.tile(idx4.reshape(B, 4, N // 16, 16).transpose(0, 1, 3, 2),
                       (1, 1, 8, 1))                                          # [B,4,128,N/16]
    bw_wrap = w4.reshape(B, 4, NJ, 128).transpose(0, 1, 3, 2)                 # [B,4,128,NJ]
    bw_wrap = np.ascontiguousarray(bw_wrap)
    idx_wrap = np.ascontiguousarray(idx_wrap)

    xyT = np.ascontiguousarray(bp.transpose(0, 2, 1))                          # [B,2,N]
    maskN = mask[:, 0, :, 0]                                                   # [B,N]
    mask2 = np.ascontiguousarray(np.stack([maskN, maskN], axis=1))             # [B,2,N]

    wfirst = np.ascontiguousarray(first_W.transpose(0, 2, 1))                  # [2,130,S]
    wmid = np.ascontiguousarray(mid_W.transpose(0, 1, 3, 2))                   # [6,4,S,S]
    wlast = np.ascontiguousarray(last_W.transpose(0, 2, 1))                    # [2,S,S]
    wfc = np.ascontiguousarray(fc_W.T)                                         # [S,2]

    bsum = np.empty((NG, S), np.float32)
    bsum[0] = first_b[0] + first_b[1]
    for i in range(NMID):
        bsum[1 + 2 * i] = mid_b[i, 0] + mid_b[i, 1]
        bsum[2 + 2 * i] = mid_b[i, 2] + mid_b[i, 3]
    bsum[NG - 1] = last_b[0] + last_b[1]
    biases = np.ascontiguousarray(bsum.reshape(NG, 2, 128).transpose(0, 2, 1))  # [NG,128,2]
    fcb = np.ascontiguousarray(fc_b.reshape(2, 1))
    ones2 = np.ones((2, 1), np.float32)

    in_maps = []
    for c in range(NCORES):
        s = slice(c * PER, (c + 1) * PER)
        in_maps.append({
            "featT": featT[s], "adjT": adjT[s], "idx": idx_wrap[s],
            "bw": bw_wrap[s], "xyT": xyT[s], "mask2": mask2[s],
            "wfirst": wfirst, "wmid": wmid, "wlast": wlast, "wfc": wfc,
            "biases": biases, "fcb": fcb, "ones2": ones2,
        })
    return in_maps


_NC_CACHE = {}


def get_nc(reps: int = 0, **kw):
    key = (reps, tuple(sorted(kw.items())))
    if key not in _NC_CACHE:
        _NC_CACHE[key] = build(reps, **kw)
    return _NC_CACHE[key]


def kernel(**inputs):
    nc = get_nc(0)
    in_maps = prep_host(inputs)
    res = run_bass_kernel_spmd(nc, in_maps, list(range(NCORES)))
    energy = np.empty((B,), np.float32)
    poly = np.empty((B, N, 2), np.float32)
    pred = np.empty((B, N, 2), np.float32)
    for c in range(NCORES):
        r = res.results[c]
        for b in range(PER):
            gb = c * PER + b
            energy[gb] = r["energy"][b, 0]
            poly[gb] = r["polyT"][b].T
            pred[gb] = r["predT"][b].T
    return energy, poly, pred


# revision 33
# speedup vs baseline: 1.0717x; 1.0717x over previous
"""DeformGNN Trainium2 kernel.

Data-parallel over batch: 16 batches -> 8 cores x 2 batches each.
Per batch on device:
  - bilinear feature sampling via dma_gather from host-transposed
    features [HW, C] (indices/weights precomputed on host from base_point)
  - 14 graph convolutions h = relu(x @ Wa.T + adj @ (x @ Wb.T) + b)
    with the activation kept in transposed layout [S, N] so biases are
    per-partition, using float32r matmuls (full PE rate at N>=256)
  - fc head, mask multiply, laplacian energy
adj is pre-transposed on host (adjT[j, i] = adj[i, j]) so the PE
contraction (over j) runs on the partition axis with no on-device
transposes of the 4MB matrix.
"""
import numpy as np
import concourse.bass as bass
import concourse.tile as tile
from concourse import bacc, mybir, masks
from concourse.bass_utils import run_bass_kernel_spmd

F32 = mybir.dt.float32
F32R = mybir.dt.float32r
I16 = mybir.dt.int16

B, C, H, W, N = 16, 128, 128, 128, 1024
HW = H * W
S = 256
NMID = 6
NG = 2 + 2 * NMID  # 14 gconvs
NCORES = 8
PER = B // NCORES  # batches per core
NJ = N // 128      # 8 n-chunks
EPS2 = 2e-10       # (dx^2+eps)+(dy^2+eps) = dx^2+dy^2+2eps

ACT_F = mybir.ActivationFunctionType


def build(reps: int = 0, n_swdge: int = 4, do_gather: bool = True,
          n_blocks: int = NMID, do_tail: bool = True, timing: bool = False,
          out_eng: str = "sync", w_eng: str = "scalar",
          stream_w: bool = True, z0t_pair: bool = False):
    """Build the per-core Bass program. reps>0 wraps the batch loop in a
    hardware For_i repeat loop (for differential timing only). The other
    kwargs disable phases for profiling bisection. timing=True turns the
    bulky inputs (features/adj/weights) into uninitialized Internal DRAM so
    timing runs ship almost no data over the tunnel."""
    nc = bacc.Bacc("TRN2", num_swdge_queues=n_swdge)
    oeng = getattr(nc, out_eng)
    weng_name = w_eng
    bulk = "Internal" if timing else "ExternalInput"

    # ---- DRAM I/O (per core) ----
    featT = nc.dram_tensor("featT", [PER, HW, C], F32, kind=bulk)
    adjT_d = nc.dram_tensor("adjT", [PER, N, N], F32R, kind=bulk)
    idx_d = nc.dram_tensor("idx", [PER, 4, 128, N // 16], I16, kind="ExternalInput")
    bw_d = nc.dram_tensor("bw", [PER, 4, 128, NJ], F32, kind="ExternalInput")
    xyT_d = nc.dram_tensor("xyT", [PER, 2, N], F32R, kind="ExternalInput")
    mask2_d = nc.dram_tensor("mask2", [PER, 2, N], F32, kind="ExternalInput")
    wfirst_d = nc.dram_tensor("wfirst", [2, 130, S], F32R, kind=bulk)
    wmid_d = nc.dram_tensor("wmid", [NMID, 4, S, S], F32R, kind=bulk)
    wlast_d = nc.dram_tensor("wlast", [2, S, S], F32R, kind=bulk)
    wfc_d = nc.dram_tensor("wfc", [S, 2], F32R, kind="ExternalInput")
    bias_d = nc.dram_tensor("biases", [NG, 128, 2], F32, kind="ExternalInput")
    fcb_d = nc.dram_tensor("fcb", [2, 1], F32, kind="ExternalInput")
    ones2_d = nc.dram_tensor("ones2", [2, 1], F32R, kind="ExternalInput")

    predT_o = nc.dram_tensor("predT", [PER, 2, N], F32, kind="ExternalOutput")
    polyT_o = nc.dram_tensor("polyT", [PER, 2, N], F32, kind="ExternalOutput")
    energy_o = nc.dram_tensor("energy", [PER, 1], F32, kind="ExternalOutput")

    with tile.TileContext(nc) as tc:
        with (
            tc.tile_pool(name="const", bufs=1) as cpool,
            tc.tile_pool(name="act", bufs=3) as apool,
            tc.tile_pool(name="z1p", bufs=1) as zpool,
            tc.tile_pool(name="gat", bufs=1) as gpool,
            tc.tile_pool(name="cnT", bufs=2) as cnpool,
            tc.tile_pool(name="adj", bufs=(2 if stream_w else 1)) as adjpool,
            tc.tile_pool(name="wst", bufs=2) as wpool,
            tc.tile_pool(name="sml", bufs=1) as spool,
            tc.tile_pool(name="hps", bufs=1, space="PSUM") as hpool,
            tc.tile_pool(name="zps", bufs=2, space="PSUM") as zppool,
            tc.tile_pool(name="tps", bufs=2, space="PSUM") as tppool,
        ):
            # ---- constants & weights (once) ----
            ident = cpool.tile([128, 128], F32)
            masks.make_identity(nc, ident[:])
            eps_t = cpool.tile([1, 1], F32)
            nc.gpsimd.memset(eps_t[:], EPS2)
            ones2 = cpool.tile([2, 1], F32R)
            nc.sync.dma_start(ones2[:], ones2_d[:])
            fcb_t = cpool.tile([2, 1], F32)
            nc.sync.dma_start(fcb_t[:], fcb_d[:])
            bias_t = cpool.tile([128, NG, 2], F32)
            nc.sync.dma_start(bias_t[:], bias_d[:].rearrange("g p c -> p g c"))

            # weights ride their own HWDGE FIFO so they never block the
            # small per-batch control DMAs
            weng = getattr(nc, weng_name)
            wfa = cpool.tile([128, 2, S], F32R)  # first-layer d-rows 0:128
            weng.dma_start(wfa[:], wfirst_d[:, 0:128, :].rearrange("k p s -> p k s"))
            wfb = cpool.tile([2, 2, S], F32R)    # first-layer d-rows 128:130 (xy)
            weng.dma_start(wfb[:], wfirst_d[:, 128:130, :].rearrange("k p s -> p k s"))
            if not stream_w:
                wmid = cpool.tile([128, NMID, 4, 2, S], F32R)
                for i in range(NMID):
                    weng.dma_start(
                        wmid[:, i], wmid_d[i].rearrange("k (d p) s -> p k d s", p=128))
            wlast = cpool.tile([128, 2, 2, S], F32R)
            weng.dma_start(wlast[:], wlast_d[:].rearrange("k (d p) s -> p k d s", p=128))
            wfc = cpool.tile([128, 2, 2], F32R)
            weng.dma_start(wfc[:], wfc_d[:].rearrange("(d p) c -> p d c", p=128))

            def gconv(g, xchunks, wa_chunks, wb_chunks, outT, prevT):
                """h = x @ Wa.T + adj @ (x @ Wb.T) (+bias, +prevT residual, relu)
                xchunks: list of (AP [dp, N]) activation d-chunks (f32r)
                wa/wb_chunks: list of AP [dp, S] (f32r), same chunking
                outT: [128, 2*N] flat f32r target ([s-chunk sigma at sigma*N], [i])
                prevT: same layout or None -> relu(h+bias) ; else relu(h+bias+prevT)
                """
                nd = len(xchunks)
                # z1 = x @ Wb.T  (natural [n, s]), into SBUF f32r
                z1 = zpool.tile([128, NJ * S], F32R, tag="z1")
                for jp in range(NJ // 2):
                    zps = zppool.tile([128, 512], F32, tag="zp")
                    for jj in range(2):
                        j = 2 * jp + jj
                        for d in range(nd):
                            nc.tensor.matmul(
                                zps[:, jj * 256:(jj + 1) * 256],
                                xchunks[d][:, j * 128:(j + 1) * 128],
                                wb_chunks[d],
                                start=(d == 0), stop=(d == nd - 1))
                    eng = nc.vector if jp % 2 == 0 else nc.scalar
                    if eng is nc.vector:
                        nc.vector.tensor_copy(z1[:, jp * 512:(jp + 1) * 512], zps[:])
                    else:
                        nc.scalar.activation(z1[:, jp * 512:(jp + 1) * 512], zps[:],
                                             ACT_F.Copy)
                # hT[sigma, i] accumulation: z0T then agg
                hps = [hpool.tile([128, 512], F32, tag=f"h{si}", name=f"h{si}")
                       for si in range(4)]
                # d-outer/hh-inner keeps the two MMs sharing a stationary
                # (same wa chunk+slice) adjacent; start flags fire at d==0
                for si in range(2):
                    if z0t_pair:
                        for d in range(nd):
                            for hh in range(2):
                                nc.tensor.matmul(
                                    hps[si * 2 + hh][:],
                                    wa_chunks[d][:, si * 128:(si + 1) * 128],
                                    xchunks[d][:, hh * 512:(hh + 1) * 512],
                                    start=(d == 0), stop=False)
                    else:
                        for hh in range(2):
                            for d in range(nd):
                                nc.tensor.matmul(
                                    hps[si * 2 + hh][:],
                                    wa_chunks[d][:, si * 128:(si + 1) * 128],
                                    xchunks[d][:, hh * 512:(hh + 1) * 512],
                                    start=(d == 0), stop=False)
                for j in range(NJ):
                    # si-outer keeps the two MMs sharing a stationary adjacent
                    for si in range(2):
                        for hh in range(2):
                            nc.tensor.matmul(
                                hps[si * 2 + hh][:],
                                z1[:, j * 256 + si * 128: j * 256 + (si + 1) * 128],
                                adjT_sb[:, j, hh * 512:(hh + 1) * 512],
                                start=False, stop=(j == NJ - 1))
                # activation: outT = relu(h + bias (+prev))
                for si in range(2):
                    for hh in range(2):
                        dst = outT[:, si * N + hh * 512: si * N + (hh + 1) * 512]
                        if prevT is None:
                            nc.scalar.activation(dst, hps[si * 2 + hh][:], ACT_F.Relu,
                                                 bias=bias_t[:, g, si:si + 1])
                        else:
                            nc.vector.scalar_tensor_tensor(
                                dst, hps[si * 2 + hh][:], bias_t[:, g, si:si + 1],
                                prevT[:, si * N + hh * 512: si * N + (hh + 1) * 512],
                                op0=mybir.AluOpType.add, op1=mybir.AluOpType.add)
                            nc.scalar.activation(dst, dst, ACT_F.Relu)

            def batch_body(b):
                nonlocal adjT_sb
                # small control DMAs first on the sync FIFO, then the 4MB adjT
                idx_t = spool.tile([128, 4, N // 16], I16, tag="idx", bufs=2)
                nc.gpsimd.dma_start(idx_t[:], idx_d[b].rearrange("k p c -> p k c"))
                bw_t = spool.tile([128, 4, NJ], F32, tag="bw", bufs=2)
                nc.sync.dma_start(bw_t[:], bw_d[b].rearrange("k p c -> p k c"))
                xyT = spool.tile([2, N], F32R, tag="xyT", bufs=2)
                nc.sync.dma_start(xyT[:], xyT_d[b])
                mask2 = spool.tile([2, N], F32, tag="mask2", bufs=2)
                nc.sync.dma_start(mask2[:], mask2_d[b])
                # adjacency (transposed on host): [p(j), jchunk, i]
                adjT_sb = adjpool.tile([128, NJ, N], F32R, tag="adjT")
                nc.sync.dma_start(
                    adjT_sb[:], adjT_d[b].rearrange("(j p) i -> p j i", p=128))

                cnnT = cnpool.tile([128, N], F32R, tag="cnnT")
                if do_gather:
                    g_t = [gpool.tile([128, NJ, C], F32, tag=f"g{k}", name=f"g{k}")
                           for k in range(4)]
                    for k in range(4):
                        nc.gpsimd.dma_gather(g_t[k][:], featT[b], idx_t[:, k, :],
                                             N, N, C, queue_num=k % n_swdge)
                    # cnn[:, c, :] = sum_k bw[k][c] * g_k[:, c, :] (ping-pong A/cnn)
                    tmpA = gpool.tile([128, NJ, C], F32, tag="tmpA")
                    cnn = gpool.tile([128, NJ, C], F32, tag="cnn")
                    for cch in range(NJ):
                        nc.vector.tensor_scalar_mul(
                            tmpA[:, cch, :], g_t[0][:, cch, :], bw_t[:, 0, cch:cch + 1])
                        nc.vector.scalar_tensor_tensor(
                            cnn[:, cch, :], g_t[1][:, cch, :], bw_t[:, 1, cch:cch + 1],
                            tmpA[:, cch, :], op0=mybir.AluOpType.mult,
                            op1=mybir.AluOpType.add)
                        nc.vector.scalar_tensor_tensor(
                            tmpA[:, cch, :], g_t[2][:, cch, :], bw_t[:, 2, cch:cch + 1],
                            cnn[:, cch, :], op0=mybir.AluOpType.mult,
                            op1=mybir.AluOpType.add)
                        nc.vector.scalar_tensor_tensor(
                            cnn[:, cch, :], g_t[3][:, cch, :], bw_t[:, 3, cch:cch + 1],
                            tmpA[:, cch, :], op0=mybir.AluOpType.mult,
                            op1=mybir.AluOpType.add)
                    # transpose cnn -> cnnT [c, n] f32r
                    for half in range(2):
                        tp = tppool.tile([128, 512], F32, tag="tp")
                        for q in range(4):
                            cch = half * 4 + q
                            nc.tensor.transpose(tp[:, q * 128:(q + 1) * 128],
                                                cnn[:, cch, :], ident[:])
                        nc.scalar.activation(cnnT[:, half * 512:(half + 1) * 512],
                                             tp[:], ACT_F.Copy)
                else:
                    # profiling stand-in: any f32r-produced data
                    nc.vector.tensor_copy(cnnT[:], adjT_sb[:, 0, :])

                # ---- gconv stack (transposed activations) ----
                xA = apool.tile([128, 2 * N], F32R, tag="act")
                gconv(0, [cnnT[:], xyT[:]],
                      [wfa[:, 0], wfb[:, 0]], [wfa[:, 1], wfb[:, 1]], xA, None)
                prev = xA
                for i in range(n_blocks):
                    if stream_w:
                        wblk = wpool.tile([128, 4, 2, S], F32R, tag="wblk")
                        weng.dma_start(
                            wblk[:], wmid_d[i].rearrange("k (d p) s -> p k d s", p=128))
                    else:
                        wblk = wmid[:, i]
                    o1 = apool.tile([128, 2 * N], F32R, tag="act")
                    gconv(1 + 2 * i, [prev[:, 0:N], prev[:, N:2 * N]],
                          [wblk[:, 0, 0], wblk[:, 0, 1]],
                          [wblk[:, 1, 0], wblk[:, 1, 1]], o1, None)
                    o2 = apool.tile([128, 2 * N], F32R, tag="act")
                    gconv(2 + 2 * i, [o1[:, 0:N], o1[:, N:2 * N]],
                          [wblk[:, 2, 0], wblk[:, 2, 1]],
                          [wblk[:, 3, 0], wblk[:, 3, 1]], o2, prev)
                    prev = o2
                out = apool.tile([128, 2 * N], F32R, tag="act")
                gconv(NG - 1, [prev[:, 0:N], prev[:, N:2 * N]],
                      [wlast[:, 0, 0], wlast[:, 0, 1]],
                      [wlast[:, 1, 0], wlast[:, 1, 1]], out, None)

                if not do_tail:
                    # profiling builds: just flush something observable
                    dummy = spool.tile([2, N], F32, tag="predT", name="dummy")
                    nc.vector.tensor_copy(dummy[:], out[0:2, 0:N].bitcast(F32))
                    oeng.dma_start(predT_o[b], dummy[:])
                    return
                # ---- fc head: predT[2, N] = (out.T @ fcW.T).T + fcb ----
                predT = spool.tile([2, N], F32, tag="predT")
                for hh in range(2):
                    fps = tppool.tile([128, 512], F32, tag="tp")
                    for d in range(2):
                        nc.tensor.matmul(
                            fps[0:2, :], wfc[:, d, :],
                            out[:, d * N + hh * 512: d * N + (hh + 1) * 512],
                            start=(d == 0), stop=(d == 1))
                    nc.vector.tensor_scalar_add(
                        predT[:, hh * 512:(hh + 1) * 512], fps[0:2, :], fcb_t[:])
                oeng.dma_start(predT_o[b], predT[:])

                # delta = pred * mask ; poly = base_point + delta
                deltaT = spool.tile([2, N], F32, tag="deltaT")
                nc.vector.tensor_mul(deltaT[:], predT[:], mask2[:])
                polyT = spool.tile([2, N], F32, tag="polyT")
                nc.vector.tensor_add(polyT[:], deltaT[:], xyT[:].bitcast(F32))
                oeng.dma_start(polyT_o[b], polyT[:])

                # laplacian energy: e = mean_n sqrt(|d - adj@d|^2 + 2eps), d = delta
                tpd = tppool.tile([128, 512], F32, tag="tp")
                for cch in range(NJ):
                    nc.tensor.transpose(tpd[:, cch * 2:(cch + 1) * 2],
                                        deltaT[:, cch * 128:(cch + 1) * 128],
                                        ident[0:2, 0:2])
                dnat = spool.tile([128, NJ * 2], F32R, tag="dnat")
                nc.vector.tensor_copy(dnat[:], tpd[:, 0:NJ * 2])
                diffT = spool.tile([2, N], F32, tag="diffT")
                for hh in range(2):
                    aps = tppool.tile([128, 512], F32, tag="tp")
                    for j in range(NJ):
                        nc.tensor.matmul(
                            aps[0:2, :], dnat[:, j * 2:(j + 1) * 2],
                            adjT_sb[:, j, hh * 512:(hh + 1) * 512],
                            start=(j == 0), stop=(j == NJ - 1))
                    nc.vector.scalar_tensor_tensor(
                        diffT[:, hh * 512:(hh + 1) * 512], aps[0:2, :], -1.0,
                        deltaT[:, hh * 512:(hh + 1) * 512],
                        op0=mybir.AluOpType.mult, op1=mybir.AluOpType.add)
                sq = spool.tile([2, N], F32R, tag="sq")
                nc.vector.tensor_mul(sq[:], diffT[:], diffT[:])
                sqrtv = spool.tile([1, N], F32, tag="sqrtv")
                for hh in range(2):
                    sps = tppool.tile([128, 512], F32, tag="tp")
                    nc.tensor.matmul(sps[0:1, :], ones2[:],
                                     sq[:, hh * 512:(hh + 1) * 512])
                    nc.scalar.activation(sqrtv[:, hh * 512:(hh + 1) * 512],
                                         sps[0:1, :], ACT_F.Sqrt, bias=eps_t[:])
                esum = spool.tile([1, 1], F32, tag="esum")
                nc.vector.reduce_sum(esum[:], sqrtv[:], axis=mybir.AxisListType.X)
                en = spool.tile([1, 1], F32, tag="en")
                nc.scalar.mul(en[:], esum[:], 1.0 / N)
                oeng.dma_start(energy_o[b:b + 1, :], en[:])

            adjT_sb = None
            if reps:
                with tc.For_i(0, reps, 1):
                    for b in range(PER):
                        batch_body(b)
            else:
                for b in range(PER):
                    batch_body(b)

    nc.compile()
    return nc


# ---------------- host side ----------------

def prep_host(inputs):
    """Shard + lay out inputs for the 8 cores. Returns list of in_maps."""
    features = np.ascontiguousarray(np.asarray(inputs["features"], np.float32))
    bp = np.asarray(inputs["base_point"], np.float32)
    adj = np.asarray(inputs["base_normalized_point_adjacent"], np.float32)
    mask = np.asarray(inputs["base_point_mask"], np.float32)
    first_W = np.asarray(inputs["first_W"], np.float32)
    first_b = np.asarray(inputs["first_b"], np.float32)
    mid_W = np.asarray(inputs["mid_W"], np.float32)
    mid_b = np.asarray(inputs["mid_b"], np.float32)
    last_W = np.asarray(inputs["last_W"], np.float32)
    last_b = np.asarray(inputs["last_b"], np.float32)
    fc_W = np.asarray(inputs["fc_W"], np.float32)
    fc_b = np.asarray(inputs["fc_b"], np.float32)

    featT = np.ascontiguousarray(features.reshape(B, C, HW).transpose(0, 2, 1))
    adjT = np.ascontiguousarray(adj.transpose(0, 2, 1))

    # bilinear corners (must mirror reference._interpolated_sum exactly, f32)
    Xs = bp[:, :, 0] * np.float32(W)
    Ys = bp[:, :, 1] * np.float32(H)
    X0 = np.floor(Xs); X1 = X0 + np.float32(1)
    Y0 = np.floor(Ys); Y1 = Y0 + np.float32(1)
    w00 = (X1 - Xs) * (Y1 - Ys)
    w01 = (X1 - Xs) * (Ys - Y0)
    w10 = (Xs - X0) * (Y1 - Ys)
    w11 = (Xs - X0) * (Ys - Y0)
    X0c = np.clip(X0, 0, W - 1).astype(np.int32)
    X1c = np.clip(X1, 0, W - 1).astype(np.int32)
    Y0c = np.clip(Y0, 0, H - 1).astype(np.int32)
    Y1c = np.clip(Y1, 0, H - 1).astype(np.int32)
    idx4 = np.stack([X0c + Y0c * W, X0c + Y1c * W,
                     X1c + Y0c * W, X1c + Y1c * W], axis=1).astype(np.int16)  # [B,4,N]
    w4 = np.stack([w00, w01, w10, w11], axis=1).astype(np.float32)            # [B,4,N]
    # wrap layouts
    idx_wrap = np.tile(idx4.reshape(B, 4, N // 16, 16).transpose(0, 1, 3, 2),
                       (1, 1, 8, 1))                                          # [B,4,128,N/16]
    bw_wrap = w4.reshape(B, 4, NJ, 128).transpose(0, 1, 3, 2)                 # [B,4,128,NJ]
    bw_wrap = np.ascontiguousarray(bw_wrap)
    idx_wrap = np.ascontiguousarray(idx_wrap)

    xyT = np.ascontiguousarray(bp.transpose(0, 2, 1))                          # [B,2,N]
    maskN = mask[:, 0, :, 0]                                                   # [B,N]
    mask2 = np.ascontiguousarray(np.stack([maskN, maskN], axis=1))             # [B,2,N]

    wfirst = np.ascontiguousarray(first_W.transpose(0, 2, 1))                  # [2,130,S]
    wmid = np.ascontiguousarray(mid_W.transpose(0, 1, 3, 2))                   # [6,4,S,S]
    wlast = np.ascontiguousarray(last_W.transpose(0, 2, 1))                    # [2,S,S]
    wfc = np.ascontiguousarray(fc_W.T)                                         # [S,2]

    bsum = np.empty((NG, S), np.float32)
    bsum[0] = first_b[0] + first_b[1]
    for i in range(NMID):
        bsum[1 + 2 * i] = mid_b[i, 0] + mid_b[i, 1]
        bsum[2 + 2 * i] = mid_b[i, 2] + mid_b[i, 3]
    bsum[NG - 1] = last_b[0] + last_b[1]
    biases = np.ascontiguousarray(bsum.reshape(NG, 2, 128).transpose(0, 2, 1))  # [NG,128,2]
    fcb = np.ascontiguousarray(fc_b.reshape(2, 1))
    ones2 = np.ones((2, 1), np.float32)

    in_maps = []
    for c in range(NCORES):
        s = slice(c * PER, (c + 1) * PER)
        in_maps.append({
            "featT": featT[s], "adjT": adjT[s], "idx": idx_wrap[s],
            "bw": bw_wrap[s], "xyT": xyT[s], "mask2": mask2[s],
            "wfirst": wfirst, "wmid": wmid, "wlast": wlast, "wfc": wfc,
            "biases": biases, "fcb": fcb, "ones2": ones2,
        })
    return in_maps


_NC_CACHE = {}


def get_nc(reps: int = 0, **kw):
    key = (reps, tuple(sorted(kw.items())))
    if key not in _NC_CACHE:
        _NC_CACHE[key] = build(reps, **kw)
    return _NC_CACHE[key]


def kernel(**inputs):
    nc = get_nc(0)
    in_maps = prep_host(inputs)
    res = run_bass_kernel_spmd(nc, in_maps, list(range(NCORES)))
    energy = np.empty((B,), np.float32)
    poly = np.empty((B, N, 2), np.float32)
    pred = np.empty((B, N, 2), np.float32)
    for c in range(NCORES):
        r = res.results[c]
        for b in range(PER):
            gb = c * PER + b
            energy[gb] = r["energy"][b, 0]
            poly[gb] = r["polyT"][b].T
            pred[gb] = r["predT"][b].T
    return energy, poly, pred
